# revision 17
# baseline (speedup 1.0000x reference)
"""Trainium2 Bass kernel for CompositionalMHA (moe_routing).

Math (see reference):
  For each bank b in {q,k,v}:  proj_b = sum_{j in top4(softmax(logits_b))}
      tw_j * (x @ U_j @ V_j)
  Then 16-head causal attention over the projections, then out @ out_w.T.

Host side: the top-k selection + softmax weights depend only on the tiny
logits vectors, so they are computed here in numpy; the selected U banks are
concatenated into [d, 4*64] and the tw-scaled V banks into [4*64, d_out].

Sharding (8 cores): core c = (batch b = c//2, head-half g = c%2).
Each core gets x[b] (transposed to [d,S]), the full U-cat per bank, the
head-half columns of V-cat per bank, and the matching 512 rows of out_w.T.
It computes a partial [S, d_model] output (its 8 heads' contribution through
the output projection); the host sums the two half-contributions per batch.

Device kernel works in "transposed activation" layout [feat, S], bf16
operands with fp32 PSUM accumulation:
  hT = Ucat^T @ xT           (contract d)
  qT/kT = Vw^T @ hT          (contract 4*64)    -> [512, S]
  v    = hT^T @ Vw           (per s-tile)       -> [S, 512] (natural layout)
  scoresT[k,q] = k_h @ q_h^T per head, two heads paired per 2-bank PSUM tile
  pT = exp(scoresT*scale)    (one ACT instr per head-pair, causally trimmed)
  outT[65, q]  = [v_h | 1]^T @ pT      (row 64 = softmax denom)
  rcp = reciprocal_approx_fast(den); bc = ones ⊗ rcp (K=1 matmul broadcast)
  attnT = outT[0:64] * bc
  final[s, m] = attnT^T @ w_half       (contract feature)

A block of junk matmuls at t=0 keeps the PE busy so the HAM clock gate
reaches K=8/8 (2.4 GHz) before the real matmul stream begins.
"""

import numpy as np
import ml_dtypes

import concourse.bass as bass
import concourse.bacc as bacc
import concourse.mybir as mybir
import concourse.tile as tile
from concourse.bass_utils import run_bass_kernel_spmd

F32 = mybir.dt.float32
F32R = mybir.dt.float32r
BF16 = mybir.dt.bfloat16
AF = mybir.ActivationFunctionType

P = 128
S = 1024        # sequence length
DM = 1024       # d_model
KR = 256        # top_k * r = 4 * 64
F = 512         # features per core = 8 heads * 64
NH = 8          # heads per core
HD = 64         # head dim
NG_D = DM // P  # 8
NG_R = KR // P  # 2
NG_F = F // P   # 4
NST = S // P    # 8
NSC = S // 512  # 2

N_WARMUP = 24   # junk matmuls at t=0 to flip the HAM clock gate

TRACE = False
_cache = {}


def _emit(nc, tc, xT, us, vs, w, mask, out):
    from contextlib import ExitStack

    with ExitStack() as ctx:
        pp = ctx.enter_context(tc.tile_pool(name="persist", bufs=1))

        # ---- Phase A: warmup + input DMA ----
        scratch = pp.tile([P, 512], BF16)
        nc.gpsimd.memset(scratch, 0.0)

        xT_sb = pp.tile([P, NG_D, S], BF16)
        mask_sb = pp.tile([P, P], BF16)
        w_sb = pp.tile([P, NG_F, DM], BF16)
        u_sb = {}
        vw_sb = {}
        for b in "qkv":
            u_sb[b] = pp.tile([P, NG_D, KR], BF16, name=f"u{b}_sb")
            vw_sb[b] = pp.tile([P, NG_R, F], BF16, name=f"vw{b}_sb")

        def gdma(eng, out_sb, dram, g0, ng, cols):
            # one DMA for a [P, ng, cols] SBUF slab from a [*, cols] DRAM
            # tensor (row (g0+g)*P+p -> partition p, slot g), issued on
            # `eng`'s queue so transfers run on parallel hardware queues
            eng.dma_start(
                out=out_sb,
                in_=bass.AP(tensor=dram.tensor,
                            offset=dram.offset + g0 * P * cols,
                            ap=[[cols, P], [P * cols, ng], [1, cols]]))

        # first h-matmul needs u_v + all of xT: split those across queues
        gdma(nc.sync, u_sb["v"], us["v"], 0, NG_D, KR)
        gdma(nc.sync, xT_sb[:, 0:4, :], xT, 0, 4, S)
        gdma(nc.gpsimd, xT_sb[:, 4:8, :], xT, 4, 4, S)
        gdma(nc.gpsimd, vw_sb["v"], vs["v"], 0, NG_R, F)
        gdma(nc.scalar, u_sb["q"], us["q"], 0, NG_D, KR)
        gdma(nc.scalar, u_sb["k"], us["k"], 0, NG_D, KR)
        gdma(nc.scalar, vw_sb["q"], vs["q"], 0, NG_R, F)
        gdma(nc.scalar, vw_sb["k"], vs["k"], 0, NG_R, F)
        nc.scalar.dma_start(out=mask_sb, in_=mask)
        gdma(nc.scalar, w_sb, w, 0, NG_F, DM)

        qT_sb = pp.tile([P, NG_F, S], BF16)
        kT_sb = pp.tile([P, NG_F, S], BF16)
        vS_sb = pp.tile([P, NST, NH, HD + 1], BF16)
        nc.vector.memset(vS_sb[:, :, :, HD:HD + 1], 1.0)
        attnT_sb = pp.tile([P, NG_F, S], BF16)
        ones_sb = pp.tile([P, P], BF16)
        nc.vector.memset(ones_sb, 1.0)
        # softmax denominators: row r=(qc*2+sub) of group hp lives at
        # partition 32*r, free slot hp
        den_sb = pp.tile([P, 4, 512], F32)
        nc.vector.memset(den_sb, 1.0)
        rcp_sb = pp.tile([P, 4, 512], F32)
        rcp_bf = pp.tile([P, 4, 512], BF16)

        # ---- Phase B: projections ----
        hpool = ctx.enter_context(tc.tile_pool(name="hpool", bufs=2))
        copy_flip = [0]

        def split_copy(dst, src):
            # alternate PSUM->SBUF copies between DVE and ACT
            eng = nc.vector if copy_flip[0] % 2 == 0 else nc.scalar
            copy_flip[0] += 1
            if eng is nc.vector:
                nc.vector.tensor_copy(dst, src)
            else:
                nc.scalar.copy(out=dst, in_=src)

        with tc.tile_pool(name="pph", bufs=6, space="PSUM") as pph:
            # warmup: junk matmuls, no data deps, keeps PE busy from t=0
            junk_ps = pph.tile([P, 512], F32, name="junk_ps", tag="h_ps")
            for i in range(N_WARMUP):
                nc.tensor.matmul(junk_ps, lhsT=scratch[:, 0:P], rhs=scratch,
                                 start=True, stop=True)

            hT = {}
            for b in "vqk":
                hT[b] = hpool.tile([P, NG_R, S], BF16, name=f"hT_{b}", tag="hT")
                for mi in range(NG_R):
                    for sc in range(NSC):
                        h_ps = pph.tile([P, 512], F32, name="h_ps", tag="h_ps")
                        for g in range(NG_D):
                            nc.tensor.matmul(
                                h_ps,
                                lhsT=u_sb[b][:, g, mi * P:(mi + 1) * P],
                                rhs=xT_sb[:, g, sc * 512:(sc + 1) * 512],
                                start=(g == 0), stop=(g == NG_D - 1))
                        split_copy(hT[b][:, mi, sc * 512:(sc + 1) * 512], h_ps)
                if b == "v":
                    for st in range(NST):
                        v_ps = pph.tile([P, F], F32, name="v_ps", tag="h_ps")
                        for mi in range(NG_R):
                            nc.tensor.matmul(
                                v_ps,
                                lhsT=hT[b][:, mi, st * P:(st + 1) * P],
                                rhs=vw_sb[b][:, mi, :],
                                start=(mi == 0), stop=(mi == NG_R - 1))
                        split_copy(
                            vS_sb[:, st, :, 0:HD],
                            v_ps.rearrange("p (h e) -> p h e", h=NH))
            # q/k features interleaved by head-pair so attention can start
            # as soon as fc=0 is done
            for fc in range(NG_F):
                for b in "qk":
                    dst = qT_sb if b == "q" else kT_sb
                    for sc in range(NSC):
                        b_ps = pph.tile([P, 512], F32, name="b_ps", tag="h_ps")
                        for mi in range(NG_R):
                            nc.tensor.matmul(
                                b_ps,
                                lhsT=vw_sb[b][:, mi, fc * P:(fc + 1) * P],
                                rhs=hT[b][:, mi, sc * 512:(sc + 1) * 512],
                                start=(mi == 0), stop=(mi == NG_R - 1))
                        split_copy(dst[:, fc, sc * 512:(sc + 1) * 512], b_ps)

        # ---- Phase C: attention ----
        # Per head-pair: a qc=1 pass (kt 0..7) then a qc=0 pass (kt 0..3).
        # Each pass keeps one [P,2,512] score tile per kt (double-buffered,
        # 2x2 banks) plus two [65,512] o-accumulators (2 banks): 6 of 8
        # banks live, so score matmuls for kt+1 overlap the exp of kt.
        spp = ctx.enter_context(tc.tile_pool(name="spp", bufs=4))
        mask_flip = [0]
        with (
            tc.tile_pool(name="pps", bufs=2, space="PSUM") as pps,
            tc.tile_pool(name="ppo", bufs=4, space="PSUM") as ppo,
        ):
            for hp in range(NH // 2):
                for qc in (1, 0):
                    kts = range(NST) if qc == 1 else range(4)
                    last = NST - 1 if qc == 1 else 3
                    o_ps = {}
                    for sub in range(2):
                        o_ps[sub] = ppo.tile(
                            [HD + 1, 512], F32, name=f"o_{hp}_{qc}_{sub}",
                            tag="o_ps")
                    for kt in kts:
                        rel = P * kt - 512 * qc
                        c0 = max(rel, 0)
                        s_pair = pps.tile([P, 2, 512], F32,
                                          name=f"s_{hp}_{kt}_{qc}", tag="s_pair")
                        # dependency-free duplicate matmuls at the pass
                        # boundary keep the PE busy (HAM stays at 8/8)
                        # while the previous pass's tail drains
                        if kt == 0:
                            for _ in range(3 if qc == 1 else 1):
                                nc.tensor.matmul(
                                    s_pair[:, 0, c0:512],
                                    lhsT=kT_sb[0:HD, hp, 0:P],
                                    rhs=qT_sb[0:HD, hp,
                                              qc * 512 + c0:(qc + 1) * 512],
                                    start=True, stop=True)
                        for sub in range(2):
                            po = HD * sub
                            nc.tensor.matmul(
                                s_pair[:, sub, c0:512],
                                lhsT=kT_sb[po:po + HD, hp, kt * P:(kt + 1) * P],
                                rhs=qT_sb[po:po + HD, hp,
                                          qc * 512 + c0:(qc + 1) * 512],
                                start=True, stop=True)
                        pt = spp.tile([P, 2, 512], BF16,
                                      name=f"p_{hp}_{kt}_{qc}", tag="pT")
                        nc.scalar.activation(
                            out=pt[:, :, c0:512], in_=s_pair[:, :, c0:512],
                            func=AF.Exp, scale=0.125)
                        if 0 <= rel <= 384:
                            # diagonal 128-block: triangular causal mask,
                            # alternating DVE mul / GpSimd affine_select
                            for sub in range(2):
                                if mask_flip[0] % 2 == 0:
                                    nc.vector.tensor_mul(
                                        pt[:, sub, rel:rel + P],
                                        pt[:, sub, rel:rel + P], mask_sb)
                                else:
                                    nc.gpsimd.affine_select(
                                        out=pt[:, sub, rel:rel + P],
                                        in_=pt[:, sub, rel:rel + P],
                                        compare_op=mybir.AluOpType.is_ge,
                                        fill=0.0, base=0,
                                        pattern=[[1, P]],
                                        channel_multiplier=-1)
                                mask_flip[0] += 1
                        for sub in range(2):
                            h = 2 * hp + sub
                            nc.tensor.matmul(
                                o_ps[sub][:, c0:512],
                                lhsT=vS_sb[:, kt, h, :],
                                rhs=pt[:, sub, c0:512],
                                start=(kt == 0), stop=(kt == last))
                    for sub in range(2):
                        r = qc * 2 + sub
                        nc.vector.tensor_copy(
                            attnT_sb[HD * sub:HD * (sub + 1), hp,
                                     qc * 512:(qc + 1) * 512],
                            o_ps[sub][0:HD, :])
                        nc.vector.tensor_copy(
                            den_sb[32 * r:32 * r + 1, hp, :],
                            o_ps[sub][HD:HD + 1, :])
                nc.vector.reciprocal_approx_fast(
                    out=rcp_sb[:, hp, :], in_=den_sb[:, hp, :])
                nc.vector.tensor_copy(rcp_bf[:, hp, :], rcp_sb[:, hp, :])

        # ---- normalization + Phase D, interleaved by q-half ----
        # bc = ones (x) rcp broadcast via K=1 matmuls; normalize the qc=0
        # half of attnT, project s-tiles 0-3, then the qc=1 half, 4-7.
        spo = ctx.enter_context(tc.tile_pool(name="spo", bufs=3))
        with (
            tc.tile_pool(name="ppb", bufs=2, space="PSUM") as ppb,
            tc.tile_pool(name="ppf", bufs=5, space="PSUM") as ppf,
        ):
            def norm_rows(qc):
                for hp in range(NH // 2):
                    for sub in range(2):
                        r = qc * 2 + sub
                        bc_ps = ppb.tile([P, 512], F32,
                                         name=f"bc_{qc}_{hp}_{sub}", tag="bc")
                        nc.tensor.matmul(
                            bc_ps,
                            lhsT=ones_sb[32 * r:32 * r + 1, :],
                            rhs=rcp_bf[32 * r:32 * r + 1, hp, :],
                            start=True, stop=True,
                            tile_position=(32 * r, 0))
                        sl = attnT_sb[HD * sub:HD * (sub + 1), hp,
                                      qc * 512:(qc + 1) * 512]
                        nc.vector.tensor_mul(sl, sl, bc_ps[0:HD, :])

            def proj(st_range):
                for st in st_range:
                    for mc in range(NSC):
                        f_ps = ppf.tile([P, 512], F32, name="f_ps", tag="f_ps")
                        for fcc in range(NG_F):
                            nc.tensor.matmul(
                                f_ps,
                                lhsT=attnT_sb[:, fcc, st * P:(st + 1) * P],
                                rhs=w_sb[:, fcc, mc * 512:(mc + 1) * 512],
                                start=(fcc == 0), stop=(fcc == NG_F - 1))
                        o_sb = spo.tile([P, 512], BF16, name="o_sb", tag="o_sb")
                        split_copy(o_sb, f_ps)
                        nc.sync.dma_start(
                            out=out[st * P:(st + 1) * P,
                                    mc * 512:(mc + 1) * 512],
                            in_=o_sb)

            # dependency-free filler so the PE stays warm while the qc=0
            # normalization chain (rcp -> bc -> mul) drains
            junk2 = ppf.tile([P, 512], F32, name="junk2", tag="f_ps")
            for _ in range(12):
                nc.tensor.matmul(junk2, lhsT=scratch[:, 0:P], rhs=scratch,
                                 start=True, stop=True)
            norm_rows(0)
            proj(range(0, 4))
            norm_rows(1)
            proj(range(4, NST))


def _build():
    nc = bacc.Bacc("TRN2", target_bir_lowering=False, debug=False, num_devices=8)
    xT = nc.dram_tensor("xT", [DM, S], BF16, kind="ExternalInput").ap()
    us = {b: nc.dram_tensor(f"u{b}", [DM, KR], BF16, kind="ExternalInput").ap()
          for b in "qkv"}
    vs = {b: nc.dram_tensor(f"v{b}", [KR, F], BF16, kind="ExternalInput").ap()
          for b in "qkv"}
    w = nc.dram_tensor("w", [F, DM], BF16, kind="ExternalInput").ap()
    mask = nc.dram_tensor("mask", [P, P], BF16, kind="ExternalInput").ap()
    out = nc.dram_tensor("out", [S, DM], BF16, kind="ExternalOutput").ap()
    with tile.TileContext(nc) as tc:
        _emit(nc, tc, xT, us, vs, w, mask, out)
    nc.compile()
    return nc


def _tri_mask():
    # tri[rk, c] = 1.0 iff c >= rk  (keep where key index <= query index
    # within a diagonal 128x128 block)
    rk = np.arange(P)[:, None]
    c = np.arange(P)[None, :]
    return (c >= rk).astype(ml_dtypes.bfloat16)


def _select_bank(U, V, logits, top_k):
    lg = np.asarray(logits, np.float32)
    e = np.exp(lg - lg.max())
    wsoft = (e / e.sum()).astype(np.float32)
    ti = np.argsort(-wsoft, kind="stable")[:top_k]
    tw = wsoft[ti]
    tw = tw / tw.sum()
    Ucat = np.concatenate([U[i] for i in ti], axis=1)          # [d, k*r]
    Vcat = np.concatenate([tw[k] * V[ti[k]] for k in range(top_k)], axis=0)
    return (np.ascontiguousarray(Ucat).astype(ml_dtypes.bfloat16),
            np.ascontiguousarray(Vcat).astype(ml_dtypes.bfloat16))


def kernel(**inputs):
    x = np.asarray(inputs["x"], np.float32)          # [4, S, d]
    out_w = np.asarray(inputs["out_w"], np.float32)  # [d, d]
    top_k = int(np.asarray(inputs["top_k"]))
    assert top_k * 64 == KR, f"kernel compiled for top_k=4, got {top_k}"
    B = x.shape[0]

    cats = {}
    for b in "qkv":
        cats[b] = _select_bank(
            np.asarray(inputs[f"{b}_U"], np.float32),
            np.asarray(inputs[f"{b}_V"], np.float32),
            inputs[f"{b}_logits"], top_k)

    if "nc" not in _cache:
        _cache["nc"] = _build()
    nc = _cache["nc"]

    mask = _tri_mask()
    wT = np.ascontiguousarray(out_w.T).astype(ml_dtypes.bfloat16)  # [feat, dm]
    in_maps = []
    for c in range(8):
        b, g = c // 2, c % 2
        m = {"xT": np.ascontiguousarray(x[b].T).astype(ml_dtypes.bfloat16),
             "mask": mask,
             "w": np.ascontiguousarray(wT[g * F:(g + 1) * F, :])}
        for bank in "qkv":
            Ucat, Vcat = cats[bank]
            m[f"u{bank}"] = Ucat
            m[f"v{bank}"] = np.ascontiguousarray(Vcat[:, g * F:(g + 1) * F])
        in_maps.append(m)

    res = run_bass_kernel_spmd(nc, in_maps, core_ids=list(range(8)), trace=TRACE)
    if TRACE:
        _cache["last_results"] = res
    parts = [np.asarray(r["out"], np.float32) for r in res.results]
    full = np.stack([parts[2 * b] + parts[2 * b + 1] for b in range(B)])
    return full.astype(np.float32)


# revision 22
# speedup vs baseline: 1.0637x; 1.0637x over previous
"""Trainium2 Bass kernel for CompositionalMHA (moe_routing).

Math (see reference):
  For each bank b in {q,k,v}:  proj_b = sum_{j in top4(softmax(logits_b))}
      tw_j * (x @ U_j @ V_j)
  Then 16-head causal attention over the projections, then out @ out_w.T.

Host side: the top-k selection + softmax weights depend only on the tiny
logits vectors, so they are computed here in numpy; the selected U banks are
concatenated into [d, 4*64] and the tw-scaled V banks into [4*64, d_out].

Sharding (8 cores): core c = (batch b = c//2, head-half g = c%2).
Each core gets x[b] (transposed to [d,S]), the full U-cat per bank, the
head-half columns of V-cat per bank, and the matching 512 rows of out_w.T.
It computes a partial [S, d_model] output (its 8 heads' contribution through
the output projection); the host sums the two half-contributions per batch.

Device kernel works in "transposed activation" layout [feat, S], bf16
operands with fp32 PSUM accumulation:
  hT = Ucat^T @ xT           (contract d)
  qT/kT = Vw^T @ hT          (contract 4*64)    -> [512, S]
  v    = hT^T @ Vw           (per s-tile)       -> [S, 512] (natural layout)
  scoresT[k,q] = k_h @ q_h^T per head, two heads paired per 2-bank PSUM tile
  pT = exp(scoresT*scale)    (one ACT instr per head-pair, causally trimmed)
  outT[65, q]  = [v_h | 1]^T @ pT      (row 64 = softmax denom)
  rcp = reciprocal_approx_fast(den); bc = ones ⊗ rcp (K=1 matmul broadcast)
  attnT = outT[0:64] * bc
  final[s, m] = attnT^T @ w_half       (contract feature)

A block of junk matmuls at t=0 keeps the PE busy so the HAM clock gate
reaches K=8/8 (2.4 GHz) before the real matmul stream begins.
"""

import numpy as np
import ml_dtypes

import concourse.bass as bass
import concourse.bacc as bacc
import concourse.mybir as mybir
import concourse.tile as tile
from concourse.bass_utils import run_bass_kernel_spmd

F32 = mybir.dt.float32
F32R = mybir.dt.float32r
BF16 = mybir.dt.bfloat16
AF = mybir.ActivationFunctionType

P = 128
S = 1024        # sequence length
DM = 1024       # d_model
KR = 256        # top_k * r = 4 * 64
F = 512         # features per core = 8 heads * 64
NH = 8          # heads per core
HD = 64         # head dim
NG_D = DM // P  # 8
NG_R = KR // P  # 2
NG_F = F // P   # 4
NST = S // P    # 8
NSC = S // 512  # 2

N_WARMUP = 44   # junk matmuls at t=0 to flip the HAM clock gate

TRACE = False
_cache = {}


def _emit(nc, tc, xT, us, vs, w, mask, out):
    from contextlib import ExitStack

    with ExitStack() as ctx:
        pp = ctx.enter_context(tc.tile_pool(name="persist", bufs=1))

        # ---- Phase A: warmup + input DMA ----
        scratch = pp.tile([P, 512], BF16)
        nc.gpsimd.memset(scratch, 0.0)

        xT_sb = pp.tile([P, NG_D, S], BF16)
        mask_sb = pp.tile([P, P], BF16)
        w_sb = pp.tile([P, NG_F, DM], BF16)
        u_sb = {}
        vw_sb = {}
        for b in "qkv":
            u_sb[b] = pp.tile([P, NG_D, KR], BF16, name=f"u{b}_sb")
            vw_sb[b] = pp.tile([P, NG_R, F], BF16, name=f"vw{b}_sb")

        def gdma(eng, out_sb, dram, g0, ng, cols):
            # one DMA for a [P, ng, cols] SBUF slab from a [*, cols] DRAM
            # tensor (row (g0+g)*P+p -> partition p, slot g), issued on
            # `eng`'s queue so transfers run on parallel hardware queues
            eng.dma_start(
                out=out_sb,
                in_=bass.AP(tensor=dram.tensor,
                            offset=dram.offset + g0 * P * cols,
                            ap=[[cols, P], [P * cols, ng], [1, cols]]))

        # first h-matmul needs u_v + all of xT: split those across queues
        gdma(nc.sync, u_sb["v"], us["v"], 0, NG_D, KR)
        gdma(nc.sync, xT_sb[:, 0:3, :], xT, 0, 3, S)
        gdma(nc.gpsimd, xT_sb[:, 3:6, :], xT, 3, 3, S)
        gdma(nc.scalar, xT_sb[:, 6:8, :], xT, 6, 2, S)
        gdma(nc.gpsimd, vw_sb["v"], vs["v"], 0, NG_R, F)
        gdma(nc.scalar, u_sb["q"], us["q"], 0, NG_D, KR)
        gdma(nc.scalar, u_sb["k"], us["k"], 0, NG_D, KR)
        gdma(nc.scalar, vw_sb["q"], vs["q"], 0, NG_R, F)
        gdma(nc.scalar, vw_sb["k"], vs["k"], 0, NG_R, F)
        nc.scalar.dma_start(out=mask_sb, in_=mask)
        gdma(nc.scalar, w_sb, w, 0, NG_F, DM)

        qT_sb = pp.tile([P, NG_F, S], BF16)
        kT_sb = pp.tile([P, NG_F, S], BF16)
        vS_sb = pp.tile([P, NST, NH, HD + 1], BF16)
        nc.vector.memset(vS_sb[:, :, :, HD:HD + 1], 1.0)
        attnT_sb = pp.tile([P, NG_F, S], BF16)
        ones_sb = pp.tile([P, P], BF16)
        nc.vector.memset(ones_sb, 1.0)
        # softmax denominators: row r=(qc*2+sub) of group hp lives at
        # partition 32*r, free slot hp
        den_sb = pp.tile([P, 4, 512], F32)
        nc.vector.memset(den_sb, 1.0)
        rcp_sb = pp.tile([P, 4, 512], F32)
        rcp_bf = pp.tile([P, 4, 512], BF16)

        # ---- Phase B: projections ----
        hpool = ctx.enter_context(tc.tile_pool(name="hpool", bufs=2))
        copy_flip = [0]

        def split_copy(dst, src):
            # alternate PSUM->SBUF copies between DVE and ACT
            eng = nc.vector if copy_flip[0] % 2 == 0 else nc.scalar
            copy_flip[0] += 1
            if eng is nc.vector:
                nc.vector.tensor_copy(dst, src)
            else:
                nc.scalar.copy(out=dst, in_=src)

        # one PSUM bank stays reserved for dependency-free junk matmuls:
        # issued at phase boundaries they keep the PE busy so the HAM clock
        # gate never drops back to 4/8 while a cross-engine tail drains
        pjunk = ctx.enter_context(tc.tile_pool(name="pjunk", bufs=1,
                                               space="PSUM"))
        junk_ps = pjunk.tile([P, 512], F32, name="junk_ps")

        def pe_filler(n):
            for _ in range(n):
                nc.tensor.matmul(junk_ps, lhsT=scratch[:, 0:P], rhs=scratch,
                                 start=True, stop=True)

        with tc.tile_pool(name="pph", bufs=6, space="PSUM") as pph:
            # warmup: junk matmuls, no data deps, keeps PE busy from t=0
            pe_filler(N_WARMUP)

            hT = {}
            for b in "vqk":
                hT[b] = hpool.tile([P, NG_R, S], BF16, name=f"hT_{b}", tag="hT")
                for mi in range(NG_R):
                    for sc in range(NSC):
                        h_ps = pph.tile([P, 512], F32, name="h_ps", tag="h_ps")
                        for g in range(NG_D):
                            nc.tensor.matmul(
                                h_ps,
                                lhsT=u_sb[b][:, g, mi * P:(mi + 1) * P],
                                rhs=xT_sb[:, g, sc * 512:(sc + 1) * 512],
                                start=(g == 0), stop=(g == NG_D - 1))
                        split_copy(hT[b][:, mi, sc * 512:(sc + 1) * 512], h_ps)
                if b == "v":
                    for st in range(NST):
                        v_ps = pph.tile([P, F], F32, name="v_ps", tag="h_ps")
                        for mi in range(NG_R):
                            nc.tensor.matmul(
                                v_ps,
                                lhsT=hT[b][:, mi, st * P:(st + 1) * P],
                                rhs=vw_sb[b][:, mi, :],
                                start=(mi == 0), stop=(mi == NG_R - 1))
                        split_copy(
                            vS_sb[:, st, :, 0:HD],
                            v_ps.rearrange("p (h e) -> p h e", h=NH))
            # q/k features interleaved by head-pair so attention can start
            # as soon as fc=0 is done
            for fc in range(NG_F):
                for b in "qk":
                    dst = qT_sb if b == "q" else kT_sb
                    for sc in range(NSC):
                        b_ps = pph.tile([P, 512], F32, name="b_ps", tag="h_ps")
                        for mi in range(NG_R):
                            nc.tensor.matmul(
                                b_ps,
                                lhsT=vw_sb[b][:, mi, fc * P:(fc + 1) * P],
                                rhs=hT[b][:, mi, sc * 512:(sc + 1) * 512],
                                start=(mi == 0), stop=(mi == NG_R - 1))
                        split_copy(dst[:, fc, sc * 512:(sc + 1) * 512], b_ps)

        # ---- Phase C: attention ----
        # Per head-pair: a qc=1 pass (kt 0..7) then a qc=0 pass (kt 0..3).
        # Each pass keeps one [P,2,512] score tile per kt (double-buffered,
        # 2x2 banks) plus two [65,512] o-accumulators (2 banks): 6 of 8
        # banks live, so score matmuls for kt+1 overlap the exp of kt.
        spp = ctx.enter_context(tc.tile_pool(name="spp", bufs=4))
        mask_flip = [0]
        with (
            tc.tile_pool(name="pps", bufs=2, space="PSUM") as pps,
            tc.tile_pool(name="ppo", bufs=3, space="PSUM") as ppo,
        ):
            for hp in range(NH // 2):
                for qc in (1, 0):
                    kts = range(NST) if qc == 1 else range(4)
                    last = NST - 1 if qc == 1 else 3
                    pe_filler(5)
                    o_ps = {}
                    for sub in range(2):
                        o_ps[sub] = ppo.tile(
                            [HD + 1, 512], F32, name=f"o_{hp}_{qc}_{sub}",
                            tag="o_ps")
                    for kt in kts:
                        rel = P * kt - 512 * qc
                        c0 = max(rel, 0)
                        s_pair = pps.tile([P, 2, 512], F32,
                                          name=f"s_{hp}_{kt}_{qc}", tag="s_pair")
                        for sub in range(2):
                            po = HD * sub
                            nc.tensor.matmul(
                                s_pair[:, sub, c0:512],
                                lhsT=kT_sb[po:po + HD, hp, kt * P:(kt + 1) * P],
                                rhs=qT_sb[po:po + HD, hp,
                                          qc * 512 + c0:(qc + 1) * 512],
                                start=True, stop=True)
                        pt = spp.tile([P, 2, 512], BF16,
                                      name=f"p_{hp}_{kt}_{qc}", tag="pT")
                        nc.scalar.activation(
                            out=pt[:, :, c0:512], in_=s_pair[:, :, c0:512],
                            func=AF.Exp, scale=0.125)
                        if 0 <= rel <= 384:
                            # diagonal 128-block: triangular causal mask,
                            # alternating DVE mul / GpSimd affine_select
                            for sub in range(2):
                                if mask_flip[0] % 2 == 0:
                                    nc.vector.tensor_mul(
                                        pt[:, sub, rel:rel + P],
                                        pt[:, sub, rel:rel + P], mask_sb)
                                else:
                                    nc.gpsimd.affine_select(
                                        out=pt[:, sub, rel:rel + P],
                                        in_=pt[:, sub, rel:rel + P],
                                        compare_op=mybir.AluOpType.is_ge,
                                        fill=0.0, base=0,
                                        pattern=[[1, P]],
                                        channel_multiplier=-1)
                                mask_flip[0] += 1
                        for sub in range(2):
                            h = 2 * hp + sub
                            nc.tensor.matmul(
                                o_ps[sub][:, c0:512],
                                lhsT=vS_sb[:, kt, h, :],
                                rhs=pt[:, sub, c0:512],
                                start=(kt == 0), stop=(kt == last))
                    for sub in range(2):
                        r = qc * 2 + sub
                        nc.vector.tensor_copy(
                            attnT_sb[HD * sub:HD * (sub + 1), hp,
                                     qc * 512:(qc + 1) * 512],
                            o_ps[sub][0:HD, :])
                        nc.vector.tensor_copy(
                            den_sb[32 * r:32 * r + 1, hp, :],
                            o_ps[sub][HD:HD + 1, :])
                nc.vector.reciprocal_approx_fast(
                    out=rcp_sb[:, hp, :], in_=den_sb[:, hp, :])
                nc.vector.tensor_copy(rcp_bf[:, hp, :], rcp_sb[:, hp, :])

        # ---- normalization + Phase D, interleaved by q-half ----
        # bc = ones (x) rcp broadcast via K=1 matmuls; normalize the qc=0
        # half of attnT, project s-tiles 0-3, then the qc=1 half, 4-7.
        spo = ctx.enter_context(tc.tile_pool(name="spo", bufs=3))
        with (
            tc.tile_pool(name="ppb", bufs=2, space="PSUM") as ppb,
            tc.tile_pool(name="ppf", bufs=5, space="PSUM") as ppf,
        ):
            def norm_rows(qc):
                for hp in range(NH // 2):
                    for sub in range(2):
                        r = qc * 2 + sub
                        bc_ps = ppb.tile([P, 512], F32,
                                         name=f"bc_{qc}_{hp}_{sub}", tag="bc")
                        nc.tensor.matmul(
                            bc_ps,
                            lhsT=ones_sb[32 * r:32 * r + 1, :],
                            rhs=rcp_bf[32 * r:32 * r + 1, hp, :],
                            start=True, stop=True,
                            tile_position=(32 * r, 0))
                        sl = attnT_sb[HD * sub:HD * (sub + 1), hp,
                                      qc * 512:(qc + 1) * 512]
                        nc.vector.tensor_mul(sl, sl, bc_ps[0:HD, :])

            def proj(st_range):
                for st in st_range:
                    for mc in range(NSC):
                        f_ps = ppf.tile([P, 512], F32, name="f_ps", tag="f_ps")
                        for fcc in range(NG_F):
                            nc.tensor.matmul(
                                f_ps,
                                lhsT=attnT_sb[:, fcc, st * P:(st + 1) * P],
                                rhs=w_sb[:, fcc, mc * 512:(mc + 1) * 512],
                                start=(fcc == 0), stop=(fcc == NG_F - 1))
                        o_sb = spo.tile([P, 512], BF16, name="o_sb", tag="o_sb")
                        split_copy(o_sb, f_ps)
                        nc.sync.dma_start(
                            out=out[st * P:(st + 1) * P,
                                    mc * 512:(mc + 1) * 512],
                            in_=o_sb)

            # dependency-free filler so the PE stays warm while the qc=0
            # normalization chain (rcp -> bc -> mul) drains
            pe_filler(12)
            norm_rows(0)
            proj(range(0, 4))
            norm_rows(1)
            proj(range(4, NST))


def _build():
    nc = bacc.Bacc("TRN2", target_bir_lowering=False, debug=False, num_devices=8)
    xT = nc.dram_tensor("xT", [DM, S], BF16, kind="ExternalInput").ap()
    us = {b: nc.dram_tensor(f"u{b}", [DM, KR], BF16, kind="ExternalInput").ap()
          for b in "qkv"}
    vs = {b: nc.dram_tensor(f"v{b}", [KR, F], BF16, kind="ExternalInput").ap()
          for b in "qkv"}
    w = nc.dram_tensor("w", [F, DM], BF16, kind="ExternalInput").ap()
    mask = nc.dram_tensor("mask", [P, P], BF16, kind="ExternalInput").ap()
    out = nc.dram_tensor("out", [S, DM], BF16, kind="ExternalOutput").ap()
    with tile.TileContext(nc) as tc:
        _emit(nc, tc, xT, us, vs, w, mask, out)
    nc.compile()
    return nc


def _tri_mask():
    # tri[rk, c] = 1.0 iff c >= rk  (keep where key index <= query index
    # within a diagonal 128x128 block)
    rk = np.arange(P)[:, None]
    c = np.arange(P)[None, :]
    return (c >= rk).astype(ml_dtypes.bfloat16)


def _select_bank(U, V, logits, top_k):
    lg = np.asarray(logits, np.float32)
    e = np.exp(lg - lg.max())
    wsoft = (e / e.sum()).astype(np.float32)
    ti = np.argsort(-wsoft, kind="stable")[:top_k]
    tw = wsoft[ti]
    tw = tw / tw.sum()
    Ucat = np.concatenate([U[i] for i in ti], axis=1)          # [d, k*r]
    Vcat = np.concatenate([tw[k] * V[ti[k]] for k in range(top_k)], axis=0)
    return (np.ascontiguousarray(Ucat).astype(ml_dtypes.bfloat16),
            np.ascontiguousarray(Vcat).astype(ml_dtypes.bfloat16))


def kernel(**inputs):
    x = np.asarray(inputs["x"], np.float32)          # [4, S, d]
    out_w = np.asarray(inputs["out_w"], np.float32)  # [d, d]
    top_k = int(np.asarray(inputs["top_k"]))
    assert top_k * 64 == KR, f"kernel compiled for top_k=4, got {top_k}"
    B = x.shape[0]

    cats = {}
    for b in "qkv":
        cats[b] = _select_bank(
            np.asarray(inputs[f"{b}_U"], np.float32),
            np.asarray(inputs[f"{b}_V"], np.float32),
            inputs[f"{b}_logits"], top_k)

    if "nc" not in _cache:
        _cache["nc"] = _build()
    nc = _cache["nc"]

    mask = _tri_mask()
    wT = np.ascontiguousarray(out_w.T).astype(ml_dtypes.bfloat16)  # [feat, dm]
    in_maps = []
    for c in range(8):
        b, g = c // 2, c % 2
        m = {"xT": np.ascontiguousarray(x[b].T).astype(ml_dtypes.bfloat16),
             "mask": mask,
             "w": np.ascontiguousarray(wT[g * F:(g + 1) * F, :])}
        for bank in "qkv":
            Ucat, Vcat = cats[bank]
            m[f"u{bank}"] = Ucat
            m[f"v{bank}"] = np.ascontiguousarray(Vcat[:, g * F:(g + 1) * F])
        in_maps.append(m)

    res = run_bass_kernel_spmd(nc, in_maps, core_ids=list(range(8)), trace=TRACE)
    if TRACE:
        _cache["last_results"] = res
    parts = [np.asarray(r["out"], np.float32) for r in res.results]
    full = np.stack([parts[2 * b] + parts[2 * b + 1] for b in range(B)])
    return full.astype(np.float32)
